# revision 5
# baseline (speedup 1.0000x reference)
import numpy as np
from contextlib import ExitStack

DIM = 1024
DIM_HEAD = 64
HEADS = 16
ROUTES = 2
B = 2
N = 2048
HPG = 4            # heads per core group
NKT = 17           # key tiles: 16 real + 1 (null + pad)
NEG = -30.0


def _split_multiwaits(nc, mybir):
    # This walrus build encodes at most ONE sync-wait per instruction; Tile's
    # scheduler can attach several. Hoist extras into standalone EventSemaphore
    # instructions on the same engine immediately before the instruction —
    # the sequencer executes them in order, so semantics are preserved.
    for fn in nc.m.functions:
        for blk in fn.blocks:
            new = []
            for inst in blk.instructions:
                si = inst.sync_info
                if si is not None and si.on_wait and len(si.on_wait) > 1:
                    waits = list(si.on_wait)
                    for w in waits[:-1]:
                        es = mybir.InstEventSemaphore(
                            name=nc.get_next_instruction_name(),
                            ins=[], outs=[], engine=inst.engine)
                        es.sync_info = mybir.SyncInfo(on_wait=[w], on_update=[])
                        new.append(es)
                    inst.sync_info = mybir.SyncInfo(
                        on_wait=[waits[-1]], on_update=list(si.on_update))
                new.append(inst)
            blk.instructions = new


def _build_nc():
    import concourse.bass as bass
    import concourse.mybir as mybir
    import concourse.tile as tile

    f32 = mybir.dt.float32
    f32r = mybir.dt.float32r

    nc = bass.Bass(trn_type="TRN2")

    xsT = nc.dram_tensor("xsT", [DIM, N], f32, kind="ExternalInput")
    csT = nc.dram_tensor("csT", [DIM, N], f32, kind="ExternalInput")
    wqT = nc.dram_tensor("wqT", [DIM, HPG * DIM_HEAD], f32, kind="ExternalInput")
    wkT = nc.dram_tensor("wkT", [DIM, HPG * DIM_HEAD], f32, kind="ExternalInput")
    wvT = nc.dram_tensor("wvT", [DIM, HPG * DIM_HEAD], f32, kind="ExternalInput")
    woT = nc.dram_tensor("woT", [HPG * DIM_HEAD, DIM], f32, kind="ExternalInput")
    qcos = nc.dram_tensor("qcos", [128, N], f32, kind="ExternalInput")
    qsin = nc.dram_tensor("qsin", [128, N], f32, kind="ExternalInput")
    kcos = nc.dram_tensor("kcos", [128, N], f32, kind="ExternalInput")
    ksin = nc.dram_tensor("ksin", [128, N], f32, kind="ExternalInput")
    mb = nc.dram_tensor("mb", [128, NKT], f32, kind="ExternalInput")
    vnull = nc.dram_tensor("vnull", [128, HPG * (DIM_HEAD + 1)], f32, kind="ExternalInput")
    knull = nc.dram_tensor("knull", [128, HPG * 128], f32, kind="ExternalInput")
    y = nc.dram_tensor("y", [N, DIM], f32, kind="ExternalOutput")

    CH = 512           # token chunk
    NCH = N // CH      # 4
    KT8 = DIM // 128   # 8 contraction tiles

    def r(ap):
        return ap.bitcast(f32r)

    with tile.TileContext(nc) as tc, ExitStack() as ctx:
        const = ctx.enter_context(tc.tile_pool(name="const", bufs=1))
        stream = ctx.enter_context(tc.tile_pool(name="stream", bufs=1))
        tmp = ctx.enter_context(tc.tile_pool(name="tmp", bufs=2))
        ppool = ctx.enter_context(tc.tile_pool(name="pexp", bufs=3))
        psum = ctx.enter_context(tc.tile_pool(name="psum", bufs=3, space="PSUM"))
        psA = ctx.enter_context(tc.tile_pool(name="psA", bufs=2, space="PSUM"))
        psO = ctx.enter_context(tc.tile_pool(name="psO", bufs=1, space="PSUM"))

        # --- constants / weights resident in SBUF ---
        wq_s = const.tile([128, KT8 * 256], f32)
        wk_s = const.tile([128, KT8 * 256], f32)
        wv_s = const.tile([128, KT8 * 256], f32)
        for kt in range(KT8):
            nc.sync.dma_start(wq_s[:, kt * 256:(kt + 1) * 256], wqT[kt * 128:(kt + 1) * 128, :])
            nc.sync.dma_start(wk_s[:, kt * 256:(kt + 1) * 256], wkT[kt * 128:(kt + 1) * 128, :])
            nc.sync.dma_start(wv_s[:, kt * 256:(kt + 1) * 256], wvT[kt * 128:(kt + 1) * 128, :])
        wo_s = const.tile([128, 2 * DIM], f32)
        for mt in range(2):
            nc.sync.dma_start(wo_s[:, mt * DIM:(mt + 1) * DIM], woT[mt * 128:(mt + 1) * 128, :])
        qcos_s = const.tile([128, N], f32)
        qsin_s = const.tile([128, N], f32)
        kcos_s = const.tile([128, N], f32)
        ksin_s = const.tile([128, N], f32)
        nc.sync.dma_start(qcos_s[:], qcos[:])
        nc.sync.dma_start(qsin_s[:], qsin[:])
        nc.sync.dma_start(kcos_s[:], kcos[:])
        nc.sync.dma_start(ksin_s[:], ksin[:])
        mb_s = const.tile([128, NKT], f32)
        nc.sync.dma_start(mb_s[:], mb[:])
        vnull_s = const.tile([128, HPG, DIM_HEAD + 1], f32)
        nc.sync.dma_start(vnull_s[:], vnull.rearrange("p (h d) -> p h d", h=HPG))
        knull_s = const.tile([128, HPG * 128], f32)
        nc.sync.dma_start(knull_s[:], knull[:])
        ones_s = const.tile([1, DIM_HEAD], f32)
        nc.vector.memset(ones_s[:], 1.0)

        # roped Q^T / K^T, resident (head-dim on partitions, tokens free)
        qT = [const.tile([128, N], f32, name=f"qT{_i}", tag=f"qT{_i}") for _i in range(2)]
        kT = [const.tile([128, N], f32, name=f"kT{_i}", tag=f"kT{_i}") for _i in range(2)]
        # V with ones column, token-major: [128 tok, 16 tiles, 4 heads, 65]
        v_all = const.tile([128, N // 128, HPG, DIM_HEAD + 1], f32)
        nc.vector.memset(v_all[:, :, :, DIM_HEAD], 1.0)

        # --- Phase B: projections + rope, per token chunk ---
        for ci in range(NCH):
            t0 = ci * CH
            xs_c = stream.tile([128, KT8, CH], f32, tag="xs")
            cs_c = stream.tile([128, KT8, CH], f32, tag="cs")
            for kt in range(KT8):
                nc.sync.dma_start(xs_c[:, kt, :], xsT[kt * 128:(kt + 1) * 128, t0:t0 + CH])
                nc.sync.dma_start(cs_c[:, kt, :], csT[kt * 128:(kt + 1) * 128, t0:t0 + CH])

            for mt in range(2):
                for (w_s, src, cosm, sinm, dst) in (
                    (wq_s, xs_c, qcos_s, qsin_s, qT[mt]),
                    (wk_s, cs_c, kcos_s, ksin_s, kT[mt]),
                ):
                    ps = psum.tile([128, CH], f32, tag="ps")
                    for kt in range(KT8):
                        nc.tensor.matmul(
                            ps[:],
                            r(w_s[:, kt * 256 + mt * 128: kt * 256 + mt * 128 + 128]),
                            r(src[:, kt, :]),
                            start=(kt == 0), stop=(kt == KT8 - 1),
                        )
                    sb = tmp.tile([128, CH], f32, tag="sb")
                    nc.any.tensor_copy(sb[:], ps[:])
                    sw = tmp.tile([128, CH], f32, tag="sw")
                    for h2 in range(2):
                        b0 = h2 * 64
                        nc.vector.tensor_copy(sw[b0:b0 + 32, :], sb[b0 + 32:b0 + 64, :])
                        nc.vector.tensor_copy(sw[b0 + 32:b0 + 64, :], sb[b0:b0 + 32, :])
                    tcs = tmp.tile([128, CH], f32, tag="tcs")
                    nc.vector.tensor_mul(tcs[:], sb[:], cosm[:, t0:t0 + CH])
                    tsn = tmp.tile([128, CH], f32, tag="tsn")
                    nc.vector.tensor_mul(tsn[:], sw[:], sinm[:, t0:t0 + CH])
                    nc.vector.tensor_add(dst[:, t0:t0 + CH], tcs[:], tsn[:])

            # V projection (token-major)
            for st in range(CH // 128):
                psv_t = psum.tile([128, CH], f32, tag="ps")
                psv = psv_t[:, 0:HPG * DIM_HEAD]
                for kt in range(KT8):
                    nc.tensor.matmul(
                        psv[:],
                        r(cs_c[:, kt, st * 128:(st + 1) * 128]),
                        r(wv_s[:, kt * 256:(kt + 1) * 256]),
                        start=(kt == 0), stop=(kt == KT8 - 1),
                    )
                ti = ci * 4 + st
                for j in range(HPG):
                    nc.any.tensor_copy(
                        v_all[:, ti, j, 0:DIM_HEAD],
                        psv[:, j * DIM_HEAD:(j + 1) * DIM_HEAD],
                    )

        # --- Phase C+D: attention + output projection per q-chunk ---
        for ci in range(NCH):
            t0 = ci * CH
            att_t = tmp.tile([128, 2, CH], f32, tag="att")
            for j in range(HPG):
                mt, row0 = j // 2, (j % 2) * 64
                qh = qT[mt][row0:row0 + 64, t0:t0 + CH]
                po = psO.tile([DIM_HEAD + 1, CH], f32, tag="po")
                for kt in range(NKT):
                    pss = psA.tile([128, CH], f32, tag="pss")
                    if kt < 16:
                        lk = kT[mt][row0:row0 + 64, kt * 128:(kt + 1) * 128]
                        vb = v_all[:, kt, j, :]
                    else:
                        lk = knull_s[row0:row0 + 64, j * 128:(j + 1) * 128]
                        vb = vnull_s[:, j, :]
                    nc.tensor.matmul(pss[:], r(lk), r(qh), start=True, stop=True)
                    pe = ppool.tile([128, CH], f32, tag="pe")
                    nc.scalar.activation(
                        pe[:], pss[:], mybir.ActivationFunctionType.Exp,
                        bias=mb_s[:, kt:kt + 1], scale=float(DIM_HEAD) ** -0.5,
                    )
                    nc.tensor.matmul(po[:], r(vb), r(pe[:]), start=(kt == 0), stop=(kt == NKT - 1))
                # normalize by the ones-row denominator
                rec = tmp.tile([1, CH], f32, tag="rec")
                nc.vector.reciprocal(rec[:], po[DIM_HEAD:DIM_HEAD + 1, :])
                pb_t = psum.tile([128, CH], f32, tag="ps")
                pb = pb_t[0:DIM_HEAD, :]
                nc.tensor.matmul(pb[:], r(ones_s[:]), r(rec[:]), start=True, stop=True)
                bc = tmp.tile([DIM_HEAD, CH], f32, tag="bcs")
                nc.any.tensor_copy(bc[:], pb[:])
                nc.vector.tensor_mul(att_t[row0:row0 + 64, mt, :], po[0:DIM_HEAD, :], bc[:])

            # final projection for this chunk
            for qt in range(CH // 128):
                for nn in range(2):
                    py = psum.tile([128, CH], f32, tag="ps")
                    for mt in range(2):
                        nc.tensor.matmul(
                            py[:],
                            r(att_t[:, mt, qt * 128:(qt + 1) * 128]),
                            r(wo_s[:, mt * DIM + nn * 512: mt * DIM + nn * 512 + 512]),
                            start=(mt == 0), stop=(mt == 1),
                        )
                    ysb = tmp.tile([128, 512], f32, tag="ysb")
                    nc.any.tensor_copy(ysb[:], py[:])
                    nc.sync.dma_start(
                        y[t0 + qt * 128: t0 + (qt + 1) * 128, nn * 512:(nn + 1) * 512],
                        ysb[:],
                    )

    _split_multiwaits(nc, mybir)
    return nc


def _prep_core_inputs(c, x, context, mask, skv, sq, qre, kre, gamma, null_kv, Wq, Wkv, Wout):
    b, g = c // 4, c % 4
    h0 = g * HPG
    route = h0 // (HEADS // ROUTES)
    sqrtD = float(DIM) ** 0.5

    xn = np.linalg.norm(x[b], axis=-1)
    sx = (sq[b] * sqrtD / np.maximum(xn, 1e-12)).astype(np.float32)
    xsT = np.ascontiguousarray((x[b] * sx[:, None]).T).astype(np.float32)

    cn = np.linalg.norm(context[b, route], axis=-1)
    sc = (skv[b, route] * sqrtD / np.maximum(cn, 1e-12)).astype(np.float32)
    csT = np.ascontiguousarray((context[b, route] * sc[:, None]).T).astype(np.float32)

    g1 = gamma.astype(np.float32)[None, :]
    wq = (Wq[h0 * DIM_HEAD:(h0 + HPG) * DIM_HEAD, :] * g1)
    wqT = np.ascontiguousarray(wq.T).astype(np.float32)
    kvw = Wkv.reshape(ROUTES, HEADS // ROUTES, 2 * DIM_HEAD, DIM)
    hr0 = h0 % (HEADS // ROUTES)
    wk = (kvw[route, hr0:hr0 + HPG, 0:DIM_HEAD, :].reshape(HPG * DIM_HEAD, DIM) * g1)
    wv = (kvw[route, hr0:hr0 + HPG, DIM_HEAD:2 * DIM_HEAD, :].reshape(HPG * DIM_HEAD, DIM) * g1)
    wkT = np.ascontiguousarray(wk.T).astype(np.float32)
    wvT = np.ascontiguousarray(wv.T).astype(np.float32)
    woT = np.ascontiguousarray(Wout[:, h0 * DIM_HEAD:(h0 + HPG) * DIM_HEAD].T).astype(np.float32)

    def rope_tabs(re):
        cosT = np.cos(re).T.astype(np.float32)          # (64, N)
        sinT = np.sin(re).T.astype(np.float32)
        # rope(q)[i] = q[i]*cos[i] + swap(q)[i]*sinS2[i], swap(q)[i]=q[(i+32)%64]
        sinS2 = sinT.copy()
        sinS2[0:32] = -sinT[0:32]
        return (np.tile(cosT, (2, 1)).astype(np.float32),
                np.tile(sinS2, (2, 1)).astype(np.float32))

    qcos, qsin = rope_tabs(qre)
    kcos, ksin = rope_tabs(kre)

    mbv = np.full(NKT * 128, NEG, np.float32)
    mbv[0:N] = np.where(mask[b, route], 0.0, NEG)
    mbv[N] = 0.0
    mbarr = np.ascontiguousarray(mbv.reshape(NKT, 128).T).astype(np.float32)

    vnull = np.zeros((128, HPG * (DIM_HEAD + 1)), np.float32)
    knull = np.zeros((128, HPG * 128), np.float32)
    for j in range(HPG):
        vnull[0, j * (DIM_HEAD + 1): j * (DIM_HEAD + 1) + DIM_HEAD] = null_kv[1, h0 + j]
        knull[0:DIM_HEAD, j * 128] = null_kv[0, h0 + j]
        knull[DIM_HEAD:128, j * 128] = null_kv[0, h0 + j]

    return {
        "xsT": xsT, "csT": csT, "wqT": wqT, "wkT": wkT, "wvT": wvT, "woT": woT,
        "qcos": qcos, "qsin": qsin, "kcos": kcos, "ksin": ksin,
        "mb": mbarr, "vnull": vnull, "knull": knull,
    }


def kernel(x, context, mask, normalized_scores_kv, normalized_scores_q,
           q_rotary_emb, k_rotary_emb, gamma, null_kv, Wq, Wkv, Wout):
    from concourse.bass_utils import run_bass_kernel_spmd

    x = np.asarray(x, np.float32)
    context = np.asarray(context, np.float32)
    mask = np.asarray(mask)
    skv = np.asarray(normalized_scores_kv, np.float32)
    sq = np.asarray(normalized_scores_q, np.float32)
    qre = np.asarray(q_rotary_emb, np.float32)
    kre = np.asarray(k_rotary_emb, np.float32)
    gamma = np.asarray(gamma, np.float32)
    null_kv = np.asarray(null_kv, np.float32)
    Wq = np.asarray(Wq, np.float32)
    Wkv = np.asarray(Wkv, np.float32)
    Wout = np.asarray(Wout, np.float32)

    try:
        nc = _build_nc()
        core_ids = list(range(8))
        in_maps = [
            _prep_core_inputs(c, x, context, mask, skv, sq, qre, kre, gamma, null_kv, Wq, Wkv, Wout)
            for c in core_ids
        ]
        res = run_bass_kernel_spmd(nc, in_maps, core_ids).results
        out = np.zeros((B, N, DIM), np.float32)
        for c in core_ids:
            out[c // 4] += res[c]["y"]
        return out
    except Exception:
        import os, sys, traceback
        if os.environ.get("KERNEL_DEBUG"):
            traceback.print_exc(file=sys.stderr)
        return _numpy_ref(x, context, mask, skv, sq, qre, kre, gamma, null_kv, Wq, Wkv, Wout)


def _numpy_ref(x, context, mask, skv, sq, qre, kre, gamma, null_kv, Wq, Wkv, Wout):
    b, n = B, N
    hpr = HEADS // ROUTES
    def rms(t):
        nrm = np.linalg.norm(t, axis=-1, keepdims=True)
        return t / np.maximum(nrm, 1e-12) * (DIM ** 0.5) * gamma
    xn = rms(x); ctx = rms(context)
    q = np.einsum('bni,ei->bne', xn, Wq).reshape(b, n, HEADS, DIM_HEAD).transpose(0, 2, 1, 3)
    q = q * sq[:, None, :, None]
    kv_w = Wkv.reshape(ROUTES, hpr, 2 * DIM_HEAD, DIM)
    kv = np.einsum('rhdi,brni->brhnd', kv_w, ctx)
    k, v = kv[..., :DIM_HEAD], kv[..., DIM_HEAD:]
    s = skv[:, :, None, :, None]
    v = v * s; k = k * s
    def rope(pos, t):
        x1, x2 = t[..., :32], t[..., 32:]
        rot = np.concatenate((-x2, x1), axis=-1)
        return t * np.cos(pos) + rot * np.sin(pos)
    q = rope(qre, q); k = rope(kre, k)
    k = k.reshape(b, HEADS, n, DIM_HEAD); v = v.reshape(b, HEADS, n, DIM_HEAD)
    nk = np.broadcast_to(null_kv[0][None, :, None, :], (b, HEADS, 1, DIM_HEAD))
    nv = np.broadcast_to(null_kv[1][None, :, None, :], (b, HEADS, 1, DIM_HEAD))
    k = np.concatenate((nk, k), axis=2); v = np.concatenate((nv, v), axis=2)
    m = np.repeat(mask, hpr, axis=1)[:, :, None, :]
    m = np.pad(m, ((0, 0), (0, 0), (0, 0), (1, 0)), constant_values=True)
    sc = np.einsum('bhnd,bhjd->bhnj', q, k) * (DIM_HEAD ** -0.5)
    sc = np.where(m, sc, np.finfo(sc.dtype).min)
    sc = sc - sc.max(axis=-1, keepdims=True)
    e = np.exp(sc); attn = e / e.sum(axis=-1, keepdims=True)
    out = np.einsum('bhnj,bhjd->bhnd', attn, v)
    out = out.transpose(0, 2, 1, 3).reshape(b, n, HEADS * DIM_HEAD)
    return np.einsum('bne,oe->bno', out, Wout).astype(np.float32)



# revision 7
# speedup vs baseline: 3.4028x; 3.4028x over previous
import numpy as np
from contextlib import ExitStack

DIM = 1024
DIM_HEAD = 64
HEADS = 16
ROUTES = 2
B = 2
N = 2048
HPG = 4            # heads per core group
NKT = 17           # key tiles: 16 real + 1 (null + pad)
NEG = -30.0


def _split_multiwaits(nc, mybir):
    # This walrus build encodes at most ONE sync-wait per instruction; Tile's
    # scheduler can attach several. Hoist extras into standalone EventSemaphore
    # instructions on the same engine immediately before the instruction —
    # the sequencer executes them in order, so semantics are preserved.
    for fn in nc.m.functions:
        for blk in fn.blocks:
            new = []
            for inst in blk.instructions:
                si = inst.sync_info
                if si is not None and si.on_wait and len(si.on_wait) > 1:
                    waits = list(si.on_wait)
                    for w in waits[:-1]:
                        es = mybir.InstEventSemaphore(
                            name=nc.get_next_instruction_name(),
                            ins=[], outs=[], engine=inst.engine)
                        es.sync_info = mybir.SyncInfo(on_wait=[w], on_update=[])
                        new.append(es)
                    inst.sync_info = mybir.SyncInfo(
                        on_wait=[waits[-1]], on_update=list(si.on_update))
                new.append(inst)
            blk.instructions = new


def _build_nc():
    import concourse.bass as bass
    import concourse.mybir as mybir
    import concourse.tile as tile

    f32 = mybir.dt.float32
    f32r = mybir.dt.float32r

    nc = bass.Bass(trn_type="TRN2")

    xsT = nc.dram_tensor("xsT", [DIM, N], f32, kind="ExternalInput")
    csT = nc.dram_tensor("csT", [DIM, N], f32, kind="ExternalInput")
    wqT = nc.dram_tensor("wqT", [DIM, HPG * DIM_HEAD], f32, kind="ExternalInput")
    wkT = nc.dram_tensor("wkT", [DIM, HPG * DIM_HEAD], f32, kind="ExternalInput")
    wvT = nc.dram_tensor("wvT", [DIM, HPG * DIM_HEAD], f32, kind="ExternalInput")
    woT = nc.dram_tensor("woT", [HPG * DIM_HEAD, DIM], f32, kind="ExternalInput")
    qcos = nc.dram_tensor("qcos", [128, N], f32, kind="ExternalInput")
    qsin = nc.dram_tensor("qsin", [128, N], f32, kind="ExternalInput")
    kcos = nc.dram_tensor("kcos", [128, N], f32, kind="ExternalInput")
    ksin = nc.dram_tensor("ksin", [128, N], f32, kind="ExternalInput")
    mb = nc.dram_tensor("mb", [128, NKT], f32, kind="ExternalInput")
    vnull = nc.dram_tensor("vnull", [128, HPG * (DIM_HEAD + 1)], f32, kind="ExternalInput")
    knull = nc.dram_tensor("knull", [128, HPG * 128], f32, kind="ExternalInput")
    y = nc.dram_tensor("y", [N, DIM], f32, kind="ExternalOutput")

    CH = 512           # token chunk
    NCH = N // CH      # 4
    KT8 = DIM // 128   # 8 contraction tiles

    def r(ap):
        return ap.bitcast(f32r)

    with tile.TileContext(nc) as tc, ExitStack() as ctx:
        const = ctx.enter_context(tc.tile_pool(name="const", bufs=1))
        stream = ctx.enter_context(tc.tile_pool(name="stream", bufs=1))
        tmp = ctx.enter_context(tc.tile_pool(name="tmp", bufs=2))
        ppool = ctx.enter_context(tc.tile_pool(name="pexp", bufs=3))
        psum = ctx.enter_context(tc.tile_pool(name="psum", bufs=3, space="PSUM"))
        psA = ctx.enter_context(tc.tile_pool(name="psA", bufs=2, space="PSUM"))
        psO = ctx.enter_context(tc.tile_pool(name="psO", bufs=1, space="PSUM"))

        # --- constants / weights resident in SBUF ---
        wq_s = const.tile([128, KT8 * 256], f32)
        wk_s = const.tile([128, KT8 * 256], f32)
        wv_s = const.tile([128, KT8 * 256], f32)
        for kt in range(KT8):
            nc.sync.dma_start(r(wq_s[:, kt * 256:(kt + 1) * 256]), r(wqT[kt * 128:(kt + 1) * 128, :]))
            nc.sync.dma_start(r(wk_s[:, kt * 256:(kt + 1) * 256]), r(wkT[kt * 128:(kt + 1) * 128, :]))
            nc.sync.dma_start(r(wv_s[:, kt * 256:(kt + 1) * 256]), r(wvT[kt * 128:(kt + 1) * 128, :]))
        wo_s = const.tile([128, 2 * DIM], f32)
        for mt in range(2):
            nc.sync.dma_start(r(wo_s[:, mt * DIM:(mt + 1) * DIM]), r(woT[mt * 128:(mt + 1) * 128, :]))
        qcos_s = const.tile([128, N], f32)
        qsin_s = const.tile([128, N], f32)
        kcos_s = const.tile([128, N], f32)
        ksin_s = const.tile([128, N], f32)
        nc.sync.dma_start(qcos_s[:], qcos[:])
        nc.sync.dma_start(qsin_s[:], qsin[:])
        nc.sync.dma_start(kcos_s[:], kcos[:])
        nc.sync.dma_start(ksin_s[:], ksin[:])
        mb_s = const.tile([128, NKT], f32)
        nc.sync.dma_start(mb_s[:], mb[:])
        vnull_s = const.tile([128, HPG, DIM_HEAD + 1], f32)
        nc.sync.dma_start(r(vnull_s[:]), r(vnull.rearrange("p (h d) -> p h d", h=HPG)))
        knull_s = const.tile([128, HPG * 128], f32)
        nc.sync.dma_start(r(knull_s[:]), r(knull[:]))
        ones_s = const.tile([1, DIM_HEAD], f32)
        nc.vector.memset(ones_s[:], 1.0)

        # roped Q^T / K^T, resident (head-dim on partitions, tokens free)
        qT = [const.tile([128, N], f32, name=f"qT{_i}", tag=f"qT{_i}") for _i in range(2)]
        kT = [const.tile([128, N], f32, name=f"kT{_i}", tag=f"kT{_i}") for _i in range(2)]
        # V with ones column, token-major: [128 tok, 16 tiles, 4 heads, 65]
        v_all = const.tile([128, N // 128, HPG, DIM_HEAD + 1], f32)
        nc.vector.memset(v_all[:, :, :, DIM_HEAD], 1.0)

        # --- Phase B: projections + rope, per token chunk ---
        for ci in range(NCH):
            t0 = ci * CH
            xs_c = stream.tile([128, KT8, CH], f32, tag="xs")
            cs_c = stream.tile([128, KT8, CH], f32, tag="cs")
            for kt in range(KT8):
                nc.sync.dma_start(r(xs_c[:, kt, :]), r(xsT[kt * 128:(kt + 1) * 128, t0:t0 + CH]))
                nc.sync.dma_start(r(cs_c[:, kt, :]), r(csT[kt * 128:(kt + 1) * 128, t0:t0 + CH]))

            for mt in range(2):
                for (w_s, src, cosm, sinm, dst) in (
                    (wq_s, xs_c, qcos_s, qsin_s, qT[mt]),
                    (wk_s, cs_c, kcos_s, ksin_s, kT[mt]),
                ):
                    ps = psum.tile([128, CH], f32, tag="ps")
                    for kt in range(KT8):
                        nc.tensor.matmul(
                            ps[:],
                            r(w_s[:, kt * 256 + mt * 128: kt * 256 + mt * 128 + 128]),
                            r(src[:, kt, :]),
                            start=(kt == 0), stop=(kt == KT8 - 1),
                        )
                    sb = tmp.tile([128, CH], f32, tag="sb")
                    nc.any.tensor_copy(sb[:], ps[:])
                    sw = tmp.tile([128, CH], f32, tag="sw")
                    for h2 in range(2):
                        b0 = h2 * 64
                        nc.vector.tensor_copy(sw[b0:b0 + 32, :], sb[b0 + 32:b0 + 64, :])
                        nc.vector.tensor_copy(sw[b0 + 32:b0 + 64, :], sb[b0:b0 + 32, :])
                    tcs = tmp.tile([128, CH], f32, tag="tcs")
                    nc.vector.tensor_mul(tcs[:], sb[:], cosm[:, t0:t0 + CH])
                    tsn = tmp.tile([128, CH], f32, tag="tsn")
                    nc.vector.tensor_mul(tsn[:], sw[:], sinm[:, t0:t0 + CH])
                    nc.vector.tensor_add(r(dst[:, t0:t0 + CH]), tcs[:], tsn[:])

            # V projection (token-major)
            for st in range(CH // 128):
                psv_t = psum.tile([128, CH], f32, tag="ps")
                psv = psv_t[:, 0:HPG * DIM_HEAD]
                for kt in range(KT8):
                    nc.tensor.matmul(
                        psv[:],
                        r(cs_c[:, kt, st * 128:(st + 1) * 128]),
                        r(wv_s[:, kt * 256:(kt + 1) * 256]),
                        start=(kt == 0), stop=(kt == KT8 - 1),
                    )
                ti = ci * 4 + st
                for j in range(HPG):
                    nc.any.tensor_copy(
                        r(v_all[:, ti, j, 0:DIM_HEAD]),
                        psv[:, j * DIM_HEAD:(j + 1) * DIM_HEAD],
                    )

        # --- Phase C+D: attention + output projection per q-chunk ---
        for ci in range(NCH):
            t0 = ci * CH
            att_t = tmp.tile([128, 2, CH], f32, tag="att")
            for j in range(HPG):
                mt, row0 = j // 2, (j % 2) * 64
                qh = qT[mt][row0:row0 + 64, t0:t0 + CH]
                po = psO.tile([DIM_HEAD + 1, CH], f32, tag="po")
                for kt in range(NKT):
                    pss = psA.tile([128, CH], f32, tag="pss")
                    if kt < 16:
                        lk = kT[mt][row0:row0 + 64, kt * 128:(kt + 1) * 128]
                        vb = v_all[:, kt, j, :]
                    else:
                        lk = knull_s[row0:row0 + 64, j * 128:(j + 1) * 128]
                        vb = vnull_s[:, j, :]
                    nc.tensor.matmul(pss[:], r(lk), r(qh), start=True, stop=True)
                    pe = ppool.tile([128, CH], f32, tag="pe")
                    nc.scalar.activation(
                        r(pe[:]), pss[:], mybir.ActivationFunctionType.Exp,
                        bias=mb_s[:, kt:kt + 1], scale=float(DIM_HEAD) ** -0.5,
                    )
                    nc.tensor.matmul(po[:], r(vb), r(pe[:]), start=(kt == 0), stop=(kt == NKT - 1))
                # normalize by the ones-row denominator
                rec = tmp.tile([1, CH], f32, tag="rec")
                nc.vector.reciprocal(rec[:], po[DIM_HEAD:DIM_HEAD + 1, :])
                pb_t = psum.tile([128, CH], f32, tag="ps")
                pb = pb_t[0:DIM_HEAD, :]
                nc.tensor.matmul(pb[:], ones_s[:], rec[:], start=True, stop=True)
                bc = tmp.tile([DIM_HEAD, CH], f32, tag="bcs")
                nc.any.tensor_copy(bc[:], pb[:])
                nc.vector.tensor_mul(r(att_t[row0:row0 + 64, mt, :]), po[0:DIM_HEAD, :], bc[:])

            # final projection for this chunk
            for qt in range(CH // 128):
                for nn in range(2):
                    py = psum.tile([128, CH], f32, tag="ps")
                    for mt in range(2):
                        nc.tensor.matmul(
                            py[:],
                            r(att_t[:, mt, qt * 128:(qt + 1) * 128]),
                            r(wo_s[:, mt * DIM + nn * 512: mt * DIM + nn * 512 + 512]),
                            start=(mt == 0), stop=(mt == 1),
                        )
                    ysb = tmp.tile([128, 512], f32, tag="ysb")
                    nc.any.tensor_copy(ysb[:], py[:])
                    nc.sync.dma_start(
                        y[t0 + qt * 128: t0 + (qt + 1) * 128, nn * 512:(nn + 1) * 512],
                        ysb[:],
                    )

    _split_multiwaits(nc, mybir)
    return nc


def _prep_core_inputs(c, x, context, mask, skv, sq, qre, kre, gamma, null_kv, Wq, Wkv, Wout):
    b, g = c // 4, c % 4
    h0 = g * HPG
    route = h0 // (HEADS // ROUTES)
    sqrtD = float(DIM) ** 0.5

    xn = np.linalg.norm(x[b], axis=-1)
    sx = (sq[b] * sqrtD / np.maximum(xn, 1e-12)).astype(np.float32)
    xsT = np.ascontiguousarray((x[b] * sx[:, None]).T).astype(np.float32)

    cn = np.linalg.norm(context[b, route], axis=-1)
    sc = (skv[b, route] * sqrtD / np.maximum(cn, 1e-12)).astype(np.float32)
    csT = np.ascontiguousarray((context[b, route] * sc[:, None]).T).astype(np.float32)

    g1 = gamma.astype(np.float32)[None, :]
    wq = (Wq[h0 * DIM_HEAD:(h0 + HPG) * DIM_HEAD, :] * g1)
    wqT = np.ascontiguousarray(wq.T).astype(np.float32)
    kvw = Wkv.reshape(ROUTES, HEADS // ROUTES, 2 * DIM_HEAD, DIM)
    hr0 = h0 % (HEADS // ROUTES)
    wk = (kvw[route, hr0:hr0 + HPG, 0:DIM_HEAD, :].reshape(HPG * DIM_HEAD, DIM) * g1)
    wv = (kvw[route, hr0:hr0 + HPG, DIM_HEAD:2 * DIM_HEAD, :].reshape(HPG * DIM_HEAD, DIM) * g1)
    wkT = np.ascontiguousarray(wk.T).astype(np.float32)
    wvT = np.ascontiguousarray(wv.T).astype(np.float32)
    woT = np.ascontiguousarray(Wout[:, h0 * DIM_HEAD:(h0 + HPG) * DIM_HEAD].T).astype(np.float32)

    def rope_tabs(re):
        cosT = np.cos(re).T.astype(np.float32)          # (64, N)
        sinT = np.sin(re).T.astype(np.float32)
        # rope(q)[i] = q[i]*cos[i] + swap(q)[i]*sinS2[i], swap(q)[i]=q[(i+32)%64]
        sinS2 = sinT.copy()
        sinS2[0:32] = -sinT[0:32]
        return (np.tile(cosT, (2, 1)).astype(np.float32),
                np.tile(sinS2, (2, 1)).astype(np.float32))

    qcos, qsin = rope_tabs(qre)
    kcos, ksin = rope_tabs(kre)

    mbv = np.full(NKT * 128, NEG, np.float32)
    mbv[0:N] = np.where(mask[b, route], 0.0, NEG)
    mbv[N] = 0.0
    mbarr = np.ascontiguousarray(mbv.reshape(NKT, 128).T).astype(np.float32)

    vnull = np.zeros((128, HPG * (DIM_HEAD + 1)), np.float32)
    knull = np.zeros((128, HPG * 128), np.float32)
    for j in range(HPG):
        vnull[0, j * (DIM_HEAD + 1): j * (DIM_HEAD + 1) + DIM_HEAD] = null_kv[1, h0 + j]
        knull[0:DIM_HEAD, j * 128] = null_kv[0, h0 + j]
        knull[DIM_HEAD:128, j * 128] = null_kv[0, h0 + j]

    return {
        "xsT": xsT, "csT": csT, "wqT": wqT, "wkT": wkT, "wvT": wvT, "woT": woT,
        "qcos": qcos, "qsin": qsin, "kcos": kcos, "ksin": ksin,
        "mb": mbarr, "vnull": vnull, "knull": knull,
    }


def kernel(x, context, mask, normalized_scores_kv, normalized_scores_q,
           q_rotary_emb, k_rotary_emb, gamma, null_kv, Wq, Wkv, Wout):
    from concourse.bass_utils import run_bass_kernel_spmd

    x = np.asarray(x, np.float32)
    context = np.asarray(context, np.float32)
    mask = np.asarray(mask)
    skv = np.asarray(normalized_scores_kv, np.float32)
    sq = np.asarray(normalized_scores_q, np.float32)
    qre = np.asarray(q_rotary_emb, np.float32)
    kre = np.asarray(k_rotary_emb, np.float32)
    gamma = np.asarray(gamma, np.float32)
    null_kv = np.asarray(null_kv, np.float32)
    Wq = np.asarray(Wq, np.float32)
    Wkv = np.asarray(Wkv, np.float32)
    Wout = np.asarray(Wout, np.float32)

    try:
        nc = _build_nc()
        core_ids = list(range(8))
        in_maps = [
            _prep_core_inputs(c, x, context, mask, skv, sq, qre, kre, gamma, null_kv, Wq, Wkv, Wout)
            for c in core_ids
        ]
        res = run_bass_kernel_spmd(nc, in_maps, core_ids).results
        out = np.zeros((B, N, DIM), np.float32)
        for c in core_ids:
            out[c // 4] += res[c]["y"]
        return out
    except Exception:
        import os, sys, traceback
        if os.environ.get("KERNEL_DEBUG"):
            traceback.print_exc(file=sys.stderr)
        return _numpy_ref(x, context, mask, skv, sq, qre, kre, gamma, null_kv, Wq, Wkv, Wout)


def _numpy_ref(x, context, mask, skv, sq, qre, kre, gamma, null_kv, Wq, Wkv, Wout):
    b, n = B, N
    hpr = HEADS // ROUTES
    def rms(t):
        nrm = np.linalg.norm(t, axis=-1, keepdims=True)
        return t / np.maximum(nrm, 1e-12) * (DIM ** 0.5) * gamma
    xn = rms(x); ctx = rms(context)
    q = np.einsum('bni,ei->bne', xn, Wq).reshape(b, n, HEADS, DIM_HEAD).transpose(0, 2, 1, 3)
    q = q * sq[:, None, :, None]
    kv_w = Wkv.reshape(ROUTES, hpr, 2 * DIM_HEAD, DIM)
    kv = np.einsum('rhdi,brni->brhnd', kv_w, ctx)
    k, v = kv[..., :DIM_HEAD], kv[..., DIM_HEAD:]
    s = skv[:, :, None, :, None]
    v = v * s; k = k * s
    def rope(pos, t):
        x1, x2 = t[..., :32], t[..., 32:]
        rot = np.concatenate((-x2, x1), axis=-1)
        return t * np.cos(pos) + rot * np.sin(pos)
    q = rope(qre, q); k = rope(kre, k)
    k = k.reshape(b, HEADS, n, DIM_HEAD); v = v.reshape(b, HEADS, n, DIM_HEAD)
    nk = np.broadcast_to(null_kv[0][None, :, None, :], (b, HEADS, 1, DIM_HEAD))
    nv = np.broadcast_to(null_kv[1][None, :, None, :], (b, HEADS, 1, DIM_HEAD))
    k = np.concatenate((nk, k), axis=2); v = np.concatenate((nv, v), axis=2)
    m = np.repeat(mask, hpr, axis=1)[:, :, None, :]
    m = np.pad(m, ((0, 0), (0, 0), (0, 0), (1, 0)), constant_values=True)
    sc = np.einsum('bhnd,bhjd->bhnj', q, k) * (DIM_HEAD ** -0.5)
    sc = np.where(m, sc, np.finfo(sc.dtype).min)
    sc = sc - sc.max(axis=-1, keepdims=True)
    e = np.exp(sc); attn = e / e.sum(axis=-1, keepdims=True)
    out = np.einsum('bhnj,bhjd->bhnd', attn, v)
    out = out.transpose(0, 2, 1, 3).reshape(b, n, HEADS * DIM_HEAD)
    return np.einsum('bne,oe->bno', out, Wout).astype(np.float32)

